# revision 28
# baseline (speedup 1.0000x reference)
"""MoE location-expert router kernel for Trainium2 (8 NeuronCores).

Problem: out[i] = W[ptr[i] % 8] @ x[i] + b[ptr[i] % 8]
  x  [4096, 1024] f32, W [8, 32000, 1024] f32, b [8, 32000] f32 (zeros)
  out [4096, 32000] f32

Strategy (vocab / tensor-parallel sharding, W-stationary):
  - Host routes tokens: stable-sort by expert (NO padding).
  - Each of the 8 cores owns a 4000-wide slice of the vocab dim of ALL
    8 experts -> identical SPMD program on every core, perfectly load
    balanced regardless of the routing distribution.
  - Per core, per expert, per 125-wide vocab tile: the W tile
    [K=128, M=125] is the PE-stationary operand and the expert's
    tokens stream through as the moving operand in groups of <=512
    (PSUM bank limit), split evenly so every group is >=250 columns
    and the next LDWEIGHTS (~136ns) hides under the stream.
  - Streamed columns = 32 vtiles x 8 kc x 4096 tokens = 1.049M cols
    (zero padding) vs 1.21M for the padded token-stationary layout.
  - Output is produced vocab-major [4000, 4096] per core; the host
    transposes + scatters back to [4096, 32000] (host time is free).
  - W loads in 500-wide chunks (1000B DRAM runs; 125-wide tiles
    degrade to 2-byte-element descriptors, ~5x slower DMA).
  - PSUM tiles are always a full 2KB bank: tiles sharing a bank get
    serialized by Tile's bank-aware dependency tracking.
  - Three DMA rings: W on sync, x on scalar, outs on gpsimd, with one
    merged out-DMA per (expert, vtile) to keep the out ring ~50% busy.
    A shared or saturated ring head-of-line blocks prefetch, stalls
    the PE, and triggers HAM cold-clock (1.2GHz) windows.
"""

import os

import numpy as np

import concourse.bacc as bacc
import concourse.bass as bass
import concourse.mybir as mybir
import concourse.tile as tile
from concourse.bass_utils import run_bass_kernel_spmd

E = 8          # experts
D = 1024       # d_model
V = 32000      # vocab
B = 4096       # tokens
NCORES = 8
VS = V // NCORES       # vocab slice per core (4000)
KT = 128               # contraction tile (partition dim)
KC = D // KT           # 8 K-chunks
MT = 125               # vocab tile (stationary free dim -> out partitions)
NVT = VS // MT         # 32 vocab tiles per core
WSUB = 4               # vocab tiles per W DMA chunk (500-wide, 1000B runs)
GMAX = 512             # moving-group cap (PSUM bank = 512 f32)

_program_cache = {}


def _token_groups(c):
    """Split c tokens into ceil(c/512) nearly-equal groups (each >=250
    for c>512 so LDWEIGHTS stays hidden under the moving stream)."""
    if c == 0:
        return []
    ng = -(-c // GMAX)
    base = c // ng
    rem = c % ng
    sizes = [base + (1 if i < rem else 0) for i in range(ng)]
    offs = np.cumsum([0] + sizes[:-1]).tolist()
    return list(zip(offs, sizes))


def _build_program(counts):
    """Trace the SPMD Tile program for the given per-expert counts."""
    io_dt = mybir.dt.float16
    out_dt = mybir.dt.float16

    nc = bacc.Bacc("TRN2", target_bir_lowering=False, debug=False,
                   enable_asserts=False, num_devices=NCORES)

    xT = nc.dram_tensor("xT", [D, B], io_dt, kind="ExternalInput").ap()
    wT = nc.dram_tensor("wT", [E, D, VS], io_dt, kind="ExternalInput").ap()
    out = nc.dram_tensor("out", [VS, B], out_dt, kind="ExternalOutput").ap()

    # [ (kc p) m -> p kc m ] views for K-chunked loads
    xT_r = xT.rearrange("(kc p) m -> p kc m", p=KT)

    with tile.TileContext(nc) as tc:
        with (
            tc.tile_pool(name="xp", bufs=3) as xpool,
            tc.tile_pool(name="wp", bufs=8) as wpool,
            tc.tile_pool(name="op", bufs=8) as opool,
            tc.tile_pool(name="wu", bufs=1) as wupool,
            tc.tile_pool(name="ps", bufs=8, space="PSUM") as pspool,
        ):
            # PE warm-up: junk matmuls during the otherwise-idle head DMA
            # wait so the HAM clock gate un-throttles (1.2 -> 2.4 GHz)
            # before the first real matmul (~2us cold-ramp saved)
            warm = wupool.tile([KT, GMAX], io_dt, tag="warm", name="warm")
            nc.any.memset(warm[:, :], 0.0)
            wps = pspool.tile([KT, GMAX], mybir.dt.float32, tag="ps",
                              name="warmps")
            for _ in range(14):
                nc.tensor.matmul(wps[:, :], warm[:, :KT], warm[:, :],
                                 start=True, stop=True)

            off = 0
            first = True
            for e in range(E):
                c = int(counts[e])
                if c == 0:
                    continue
                groups = _token_groups(c)
                xe = xpool.tile([KT, KC, c], io_dt, tag="x")
                # split by kc-half on the scalar ring: first MMs only need
                # kc=0, and the W ring is never blocked behind x loads
                nc.scalar.dma_start(
                    out=xe[:, :KC // 2, :],
                    in_=xT_r[:, :KC // 2, off:off + c],
                )
                nc.scalar.dma_start(
                    out=xe[:, KC // 2:, :],
                    in_=xT_r[:, KC // 2:, off:off + c],
                )
                wT_e = wT[e].rearrange("(kc p) v -> p kc v", p=KT)
                for wchunk in range(NVT // WSUB):
                    # 500-wide W chunk: contiguous 1000B DRAM runs -> fast DMA
                    wt = wpool.tile([KT, KC, WSUB * MT], io_dt, tag="w")
                    if first:
                        # kc-split so the very first matmul starts ~6us
                        # earlier (it only needs the kc=0 slice)
                        nc.sync.dma_start(
                            out=wt[:, :KC // 2, :],
                            in_=wT_e[:, :KC // 2, :WSUB * MT],
                        )
                        nc.sync.dma_start(
                            out=wt[:, KC // 2:, :],
                            in_=wT_e[:, KC // 2:, :WSUB * MT],
                        )
                        first = False
                    else:
                        nc.sync.dma_start(
                            out=wt[:, :, :],
                            in_=wT_e[:, :, wchunk * WSUB * MT:
                                     (wchunk + 1) * WSUB * MT],
                        )
                    for s in range(WSUB):
                        vt = wchunk * WSUB + s
                        # full-bank psum tiles: two tiles must never share
                        # a 2KB bank or bank-aware dep tracking serializes
                        # matmuls against the other tile's drain
                        pts = [pspool.tile([MT, GMAX], mybir.dt.float32,
                                           tag="ps", name=f"ps{g}")
                               for g, (_, n) in enumerate(groups)]
                        for kc in range(KC):
                            for g, (g0, n) in enumerate(groups):
                                nc.tensor.matmul(
                                    pts[g][:, :n],
                                    wt[:, kc, s * MT:(s + 1) * MT],
                                    xe[:, kc, g0:g0 + n],
                                    start=(kc == 0), stop=(kc == KC - 1),
                                )
                        # one merged out tile + one DMA per (e, vt): keeps
                        # the gpsimd out-ring at ~50% instead of ~85% so
                        # opool never backpressures the drains
                        ot = opool.tile([MT, c], out_dt, tag="o")
                        for g, (g0, n) in enumerate(groups):
                            if (vt + g) % 2 == 0:
                                nc.vector.tensor_copy(ot[:, g0:g0 + n],
                                                      pts[g][:, :n])
                            else:
                                nc.scalar.copy(ot[:, g0:g0 + n],
                                               pts[g][:, :n])
                        nc.gpsimd.dma_start(
                            out=out[vt * MT:(vt + 1) * MT, off:off + c],
                            in_=ot[:, :],
                        )
                off += c
    nc.compile()
    return nc


def _get_program(counts):
    key = tuple(int(c) for c in counts)
    if key not in _program_cache:
        _program_cache[key] = _build_program(key)
    return _program_cache[key]


def _prepare(x, pointer_addresses, W):
    idx = (np.asarray(pointer_addresses).astype(np.int64) % E).astype(np.int32)
    counts = np.bincount(idx, minlength=E)
    order = np.argsort(idx, kind="stable")
    nc = _get_program(counts)

    x = np.asarray(x, dtype=np.float32)
    xs = x[order].astype(np.float16)          # [B, D] sorted by expert
    xT = np.ascontiguousarray(xs.T)           # [D, B]

    W = np.asarray(W)
    wts = []
    for c in range(NCORES):
        Wc = W[:, c * VS:(c + 1) * VS, :]                  # [E, VS, D] view
        WTc = np.ascontiguousarray(
            Wc.transpose(0, 2, 1)).astype(np.float16)      # [E, D, VS]
        wts.append(WTc)
    return idx, order, nc, xT, wts


def _run(x, pointer_addresses, W, b, trace=False):
    idx, order, nc, xT, wts = _prepare(x, pointer_addresses, W)
    in_maps = [{"xT": xT, "wT": wts[c]} for c in range(NCORES)]
    kw = {}
    if trace:
        kw = dict(trace=True, trace_cores=[0])
    res = run_bass_kernel_spmd(nc, in_maps, list(range(NCORES)), **kw)

    out = np.empty((B, V), dtype=np.float32)
    for c in range(NCORES):
        # res [VS, B] fp16 vocab-major -> transpose to [B, VS]
        out[order, c * VS:(c + 1) * VS] = res.results[c]["out"].T

    b = np.asarray(b)
    if b.any():
        for e in range(E):
            out[idx == e] += b[e].astype(np.float32)
    return out, res


def kernel(x, pointer_addresses, W, b):
    out, _ = _run(x, pointer_addresses, W, b, trace=False)
    return out
